# revision 22
# baseline (speedup 1.0000x reference)
"""Trainium2 Bass kernel for nn_HardwiredAttention (NRI-style GNN message passing).

Math (derived from the reference):
  adj[b,t,i,j] = 1/(||locs[b,i,t]-locs[b,j,t]|| + eps) for i!=j, 0 on diag
  out[b,:,t,:] = adj[b,t] @ hidden[b,:,t,:]          ([48,48] @ [48,128] per (b,t))

Distribution: data-parallel over batch, 2 batches per core, 8 cores, no comms.

Per-core design (v2):
  - partitions p=(s,tau), t=2*tau+s, rows p=s*50+tau (100 used).
  - pairwise chain in fp32 (exact subtract; d2 can be ~1e-8 so fp16 is unsafe):
    sub_x on DVE, sub_y on GPSIMD, squares on ACT, d2-add on DVE,
    +BIG on the 96 diag elems (tiny strided tensor_scalar), sqrt on ACT,
    +eps (ACT add / DVE ts), reciprocal_approx_fast on DVE -> fp16 adj16.
  - PE transposes [50,48] -> [48,50] per (b,s,i) into PSUM, copied into a
    block-diag fp16 lhsT [96=(s,j), (b,scol,i,tau)]; zero blocks DMA'd once.
  - matmuls lhsT[96,96] @ hid[96,128] -> fp16 PSUM, drained by fp16 2x-mode
    copies split over DVE/ACT/GPSIMD, DMA'd to HBM.
"""

import os
import sys

sys.path.insert(0, "/opt/trn_rl_repo")

import numpy as np

import bass_rust
import concourse.bass as bass
import concourse.tile as tile
from concourse import bacc, mybir
from concourse.bass_utils import run_bass_kernel_spmd

F32 = mybir.dt.float32
F16 = mybir.dt.float16
ALU = mybir.AluOpType

B, N, T, H = 16, 48, 100, 128
NCORES = 8
BL = B // NCORES          # 2 batches per core
TAU = T // 2              # 50
E = N * N                 # 2304 pair block per batch
EPS = 1e-5
BIG = 1e8                 # added to diag of d2: w_diag = 1/(1e4+eps) ~ 1e-4
IH = N // 2               # 24 i's per chunk
CH = IH * N               # 1152 free elems per chunk
PITCH = BL * E            # 4608 free elems/partition for pair tiles
LF = BL * 2 * N * TAU     # 9600 lhsT free elems/row
HF = BL * TAU * H         # 12800 hid free elems/row


def _ap(t, offset, dims):
    return bass_rust.AP(t.tensor, offset, [list(d) for d in dims])


def build_nc():
    nc = bacc.Bacc("TRN2", target_bir_lowering=False, debug=False)

    xt = nc.dram_tensor("xt", [2, 128, BL * N], F32, kind="ExternalInput")
    hid = nc.dram_tensor("hid", [128, HF], F16, kind="ExternalInput")
    ident = nc.dram_tensor("ident", [128, TAU], F16, kind="ExternalInput")
    zoff = nc.dram_tensor("zoff", [N, BL, N * TAU], F16, kind="ExternalInput")
    zrow = nc.dram_tensor("zrow", [16, LF], F16, kind="ExternalInput")
    out = nc.dram_tensor("out", [BL, 2, N, TAU, H], F16, kind="ExternalOutput")

    with tile.TileContext(nc) as tc:
        _emit(nc, tc, xt, hid, ident, zoff, zrow, out)
    nc.compile()
    return nc


def _emit(nc, tc, xt, hid, ident, zoff, zrow, out):
    with (
        tc.tile_pool(name="persist", bufs=1) as pp,
        tc.tile_pool(name="tp", bufs=2, space="PSUM") as tp_pool,
        tc.tile_pool(name="mm", bufs=3, space="PSUM") as mm_pool,
        tc.tile_pool(name="ot", bufs=4) as ot_pool,
    ):
        xt_sb = pp.tile([128, 2 * BL * N], F32, tag="xt")
        hid_sb = pp.tile([128, HF], F16, tag="hid")
        id_sb = pp.tile([128, TAU], F16, tag="id")
        dx = pp.tile([128, PITCH], F32, tag="dx")
        dy = pp.tile([128, PITCH], F32, tag="dy")
        dx2 = pp.tile([128, PITCH], F32, tag="dx2")
        dy2 = pp.tile([128, PITCH], F32, tag="dy2")
        adj16 = pp.tile([128, PITCH], F16, tag="adj16")
        lhsT = pp.tile([128, LF], F16, tag="lhsT")

        # ---- loads -------------------------------------------------------
        nc.sync.dma_start(xt_sb[:], xt.ap().rearrange("c p q -> p c q"))
        nc.sync.dma_start(hid_sb[:], hid.ap())
        nc.sync.dma_start(id_sb[:], ident.ap())
        # zero lhsT: off-diagonal blocks of data rows + the junk rows
        # (rows 48-63, 112-127) that K=128 matmuls read against hid zeros
        nc.sync.dma_start(
            _ap(lhsT[:], N * TAU,
                [[LF, N], [2 * N * TAU, BL], [1, N * TAU]]),
            zoff.ap(),
        )
        nc.sync.dma_start(
            _ap(lhsT[:], 64 * LF,
                [[LF, N], [2 * N * TAU, BL], [1, N * TAU]]),
            zoff.ap(),
        )
        nc.sync.dma_start(_ap(lhsT[:], 48 * LF, [[LF, 16], [1, LF]]), zrow.ap())
        nc.sync.dma_start(_ap(lhsT[:], 112 * LF, [[LF, 16], [1, LF]]), zrow.ap())

        # ---- helpers -----------------------------------------------------
        # free layout inside a batch block: b=0 -> (i, j) ; b=1 -> (j, i)
        def chunk_ap(t, b, i0):
            base = b * E
            if b == 0:
                return _ap(t[:], base + i0 * N, [[PITCH, 128], [1, CH]])
            return _ap(t[:], base + i0, [[PITCH, 128], [N, N], [1, IH]])

        def coord_aps(b, i0):
            # returns (xi_x, xj_x, xi_y, xj_y) matching chunk iteration order
            res = []
            for c in range(2):
                cb = c * (BL * N) + b * N
                if b == 0:
                    xi = _ap(xt_sb[:], cb + i0, [[2 * BL * N, 128], [1, IH], [0, N]])
                    xj = _ap(xt_sb[:], cb, [[2 * BL * N, 128], [0, IH], [1, N]])
                else:
                    xi = _ap(xt_sb[:], cb + i0, [[2 * BL * N, 128], [0, N], [1, IH]])
                    xj = _ap(xt_sb[:], cb, [[2 * BL * N, 128], [1, N], [0, IH]])
                res += [xi, xj]
            return res

        def diag_ap(b, i0):
            return _ap(dx[:], b * E + i0 * (N + 1), [[PITCH, 128], [N + 1, IH]])

        # ---- per-batch pipeline ------------------------------------------
        tgroups = [(g * 8, min(8, TAU - g * 8)) for g in range((TAU + 7) // 8)]

        def cp_vec(dst, src):
            nc.vector.tensor_copy(dst, src)

        def cp_act(dst, src):
            nc.scalar.copy(dst, src)

        def cp_gps(dst, src):
            nc.gpsimd.tensor_copy(dst, src)

        # GPSIMD cannot read PSUM; PSUM-sourced copies go to DVE/ACT only
        ocopy_engines = [cp_vec, cp_act]
        lcopy_engines = [cp_vec, cp_act]
        oc = 0
        lc = 0

        def stage1(b, ih):
            i0 = ih * IH
            xi_x, xj_x, xi_y, xj_y = coord_aps(b, i0)
            nc.vector.tensor_tensor(chunk_ap(dx, b, i0), xi_x, xj_x, ALU.subtract)
            nc.vector.tensor_tensor(chunk_ap(dy, b, i0), xi_y, xj_y, ALU.subtract)
            nc.scalar.square(chunk_ap(dx2, b, i0), chunk_ap(dx, b, i0))
            nc.scalar.square(chunk_ap(dy2, b, i0), chunk_ap(dy, b, i0))

        def stage2(b, ih):
            i0 = ih * IH
            cdx = chunk_ap(dx, b, i0)
            cdy = chunk_ap(dy, b, i0)
            cdx2 = chunk_ap(dx2, b, i0)
            cdy2 = chunk_ap(dy2, b, i0)
            nc.vector.tensor_tensor(cdx, cdx2, cdy2, ALU.add)   # d2 -> dx
            nc.vector.tensor_scalar_add(diag_ap(b, i0), diag_ap(b, i0), BIG)
            nc.scalar.sqrt(cdy, cdx)                            # d -> dy
            nc.vector.tensor_scalar_add(cdx2, cdy, EPS)         # d+eps -> dx2
            nc.vector.reciprocal_approx_fast(out=cdy2, in_=cdx2)
            nc.scalar.copy(chunk_ap(adj16, b, i0), cdy2)        # fp32 -> fp16

        GI = 12  # i's per PSUM transpose tile (must fit one 2KB bank)

        def transposes(b, ih):
            nonlocal lc
            i0 = ih * IH
            for s in range(2):
                for g in range(IH // GI):
                    i0g = i0 + g * GI
                    pt = tp_pool.tile([N, GI * TAU], F16, tag="tp")
                    for ii in range(GI):
                        i = i0g + ii
                        if b == 0:
                            src = adj16[s * 64 : s * 64 + TAU,
                                        i * N : (i + 1) * N]
                        else:
                            src = _ap(adj16[:], (s * 64) * PITCH + E + i,
                                      [[PITCH, TAU], [N, N]])
                        nc.tensor.transpose(
                            pt[:, ii * TAU : (ii + 1) * TAU], src,
                            id_sb[s * 64 : s * 64 + TAU, :],
                        )
                    dst = _ap(
                        lhsT[:],
                        (s * 64) * LF + b * (2 * N * TAU) + s * (N * TAU)
                        + i0g * TAU,
                        [[LF, N], [1, GI * TAU]],
                    )
                    csrc = _ap(pt[:], 0, [[GI * TAU, N], [1, GI * TAU]])
                    lcopy_engines[lc % 2](dst, csrc)
                    lc += 1

        def matmuls(b):
            nonlocal oc
            for t0, tlen in tgroups:
                mt = mm_pool.tile([2 * N, 8 * H], F32, tag="mm")
                for k in range(tlen):
                    tau = t0 + k
                    w_ap = _ap(lhsT[:], b * (2 * N * TAU) + tau,
                               [[LF, 128], [TAU, 2 * N]])
                    r_ap = _ap(hid_sb[:], b * (TAU * H) + tau * H,
                               [[HF, 128], [1, H]])
                    nc.tensor.matmul(
                        mt[:, k * H : (k + 1) * H], w_ap, r_ap,
                        start=True, stop=True,
                    )
                ot = ot_pool.tile([2 * N, 8 * H], F16, tag="ot")
                cp = ocopy_engines[oc % 2]
                oc += 1
                cp(ot[:, : tlen * H], mt[:, : tlen * H])
                dst = out[b, :, :, t0 : t0 + tlen, :].rearrange(
                    "s i t h -> (s i) (t h)"
                )
                nc.sync.dma_start(dst, ot[:, : tlen * H])

        # software-pipelined schedule: chain chunks feed transposes feed
        # matmuls; b0 matmuls overlap the b1 chain
        stage1(0, 0)
        stage1(0, 1)
        stage2(0, 0)
        transposes(0, 0)
        stage2(0, 1)
        transposes(0, 1)
        stage1(1, 0)
        matmuls(0)
        stage1(1, 1)
        stage2(1, 0)
        transposes(1, 0)
        stage2(1, 1)
        transposes(1, 1)
        matmuls(1)

# ----------------------------------------------------------------------------
# Host side
# ----------------------------------------------------------------------------

def _prep_core(locs_c, hidden_c):
    """locs_c [2,48,100,2] f32, hidden_c [2,48,100,128] f32 -> input map."""
    lc = locs_c.reshape(BL, N, TAU, 2, 2)                  # (b, n, tau, s, c)
    xt_d = lc.transpose(4, 3, 2, 0, 1).reshape(2, 2, TAU, BL * N)  # (c,s,tau,q)
    xt = np.zeros((2, 128, BL * N), dtype=np.float32)
    xt[:, 0:TAU] = xt_d[:, 0]
    xt[:, 64 : 64 + TAU] = xt_d[:, 1]
    # filler rows: spread points (x=n, y=0) so junk weights stay finite
    fill = np.tile(np.arange(N, dtype=np.float32), BL)[None, :]
    xt[0, TAU:64] = fill
    xt[0, 64 + TAU : 128] = fill
    hc = hidden_c.astype(np.float16).reshape(BL, N, TAU, 2, H)
    hjb = hc.transpose(3, 1, 0, 2, 4)                      # (s, j, b, tau, h)
    hid = np.zeros((128, HF), dtype=np.float16)
    for s in range(2):
        hid[s * 64 : s * 64 + N] = hjb[s].reshape(N, HF)
    return {"xt": xt, "hid": hid}


_IDENT = None
_ZEROS = None


def _consts():
    global _IDENT, _ZEROS
    if _IDENT is None:
        idm = np.zeros((128, TAU), dtype=np.float16)
        idm[0:TAU] = np.eye(TAU, dtype=np.float16)
        idm[64 : 64 + TAU] = np.eye(TAU, dtype=np.float16)
        _IDENT = idm
        _ZEROS = (np.zeros((N, BL, N * TAU), dtype=np.float16),
                  np.zeros((16, LF), dtype=np.float16))
    return _IDENT, _ZEROS


_NC = None
LAST_EXEC_NS = None
LAST_RES = None


def _get_nc():
    global _NC
    if _NC is None:
        _NC = build_nc()
    return _NC


def kernel(locs, hidden, rel_rec=None, rel_send=None):
    locs = np.asarray(locs, dtype=np.float32)
    hidden = np.asarray(hidden, dtype=np.float32)
    ident, (zoff, zrow) = _consts()
    in_maps = []
    for k in range(NCORES):
        m = _prep_core(locs[2 * k : 2 * k + 2], hidden[2 * k : 2 * k + 2])
        m["ident"] = ident
        m["zoff"] = zoff
        m["zrow"] = zrow
        in_maps.append(m)

    nc = _get_nc()
    import kernel as _self
    res = run_bass_kernel_spmd(nc, in_maps, list(range(NCORES)), trace=False)
    _self.LAST_RES = res
    _self.LAST_EXEC_NS = getattr(res, "exec_time_ns", None)
    outs = []
    for k in range(NCORES):
        o = res.results[k]["out"].astype(np.float32).reshape(BL, 2, N, TAU, H)
        o = o.transpose(0, 2, 3, 1, 4).reshape(BL, N, T, H)  # t = 2*tau+s
        outs.append(o)
    return np.ascontiguousarray(np.concatenate(outs, axis=0), dtype=np.float32)


if __name__ == "__main__":
    rng = np.random.default_rng(0)
    locs = rng.standard_normal((B, N, T, 2), dtype=np.float32)
    hidden = rng.standard_normal((B, N, T, H), dtype=np.float32)
    got = kernel(locs, hidden)
    x = locs[..., 0]
    y = locs[..., 1]
    d = np.sqrt((x[:, :, None] - x[:, None]) ** 2 + (y[:, :, None] - y[:, None]) ** 2)
    w = 1.0 / (d + EPS) * (1.0 - np.eye(N)[None, :, :, None])
    want = np.einsum("bijt,bjth->bith", w.astype(np.float32), hidden)
    err = np.linalg.norm(got - want) / np.linalg.norm(want)
    print("rel err vs numpy:", err)
